# revision 2
# baseline (speedup 1.0000x reference)
"""2-layer GCN (GCNConv 128->64->64, symmetric norm, self-loops) on 8 TRN2 cores.

Strategy (graph/data parallel, dst-sharded):
 - Nodes are assigned to the 8 cores by degree-sorted snake dealing, so every
   core holds 12500 nodes and tile t on every core covers the same in-degree
   stratum (SPMD: one program, per-core data).
 - Aggregation identity used: with dinv = rsqrt(1+indeg),
       out[d] = dinv[d] * sum_{e: dst=d} dinv[src_e]*h[src_e]
              + dinv[d]^2 * h[d] + b
   so the table gathered per edge is h' = dinv * h and the self-loop is just
   one more slot. Slot j of node p in tile t holds the table row index of its
   j-th neighbor (slot 0 = self, pads point at a zero row).
 - Per layer: each core computes h' for its own rows (PE matmul), cores
   AllGather the full 100352-row table, then each core gathers its edges'
   rows with [128,1]-index indirect DMAs (one slot column per call) and
   reduces per tile.
"""

import os
import sys
import types

import numpy as np

import concourse.bass as bass
import concourse.bacc as bacc
import concourse.mybir as mybir
import concourse.tile as tile
from concourse import bass_utils
from concourse.masks import make_identity

N = 100000
E = 1600000
IN_C, HID_C, OUT_C = 128, 64, 64
NCORES = 8
TPC = 98                 # tiles per core
NP = TPC * 128           # padded nodes per core = 12544
V = NCORES * NP          # table rows (8 cores' slices) = 100352
ZROW = V                 # index of the all-zero table row
VT = V + 1

last_exec_time_ns = None


def _install_trace_hook():
    """Register the NTFF profile hook missing from this container's antenv stub."""
    if "antenv.axon_hooks" in sys.modules:
        return
    try:
        from trn_agent_boot.trn_boot import _ntff_profile_via_ctypes

        hook = _ntff_profile_via_ctypes("/opt/axon/libaxon_pjrt.so")
        m = types.ModuleType("antenv.axon_hooks")
        m._hook = hook
        m.get_axon_ntff_profile_hook = lambda: m._hook
        m.set_axon_ntff_profile_hook = lambda h: setattr(m, "_hook", h)
        sys.modules["antenv.axon_hooks"] = m
        bass_utils.upload_artifacts = lambda tmpdir: tmpdir
    except Exception:
        pass


def _shard(x, edge_index):
    """Host-side sharding: node->core assignment, permutation, slot matrix."""
    src = np.asarray(edge_index[0], dtype=np.int64)
    dst = np.asarray(edge_index[1], dtype=np.int64)
    indeg = np.bincount(dst, minlength=N)

    order = np.argsort(-indeg, kind="stable")
    snake = np.array([0, 1, 2, 3, 4, 5, 6, 7, 7, 6, 5, 4, 3, 2, 1, 0])
    r = np.arange(N)
    core_of_rank = snake[r % 16]
    local_rank = (r // 16) * 2 + (r % 16 >= 8)
    core_of = np.empty(N, np.int64)
    lrank = np.empty(N, np.int64)
    core_of[order] = core_of_rank
    lrank[order] = local_rank
    pos_dev = core_of * NP + lrank  # dev table row of each original node

    # per-core degree (tile-major [128, TPC]) and slot schedule
    slots_arr = np.ones((NCORES, NP), np.int64)
    slots_arr[core_of, lrank] = 1 + indeg
    c_t = slots_arr.reshape(NCORES, TPC, 128).max(axis=(0, 2)).astype(np.int64)
    off = np.concatenate([[0], np.cumsum(c_t)]).astype(np.int64)
    slot_cols = int(off[-1])

    deg_arr = np.ones((NCORES, NP), np.float32)
    deg_arr[core_of, lrank] = (1 + indeg).astype(np.float32)
    deg_tm = deg_arr.reshape(NCORES, TPC, 128).transpose(0, 2, 1).copy()  # [NC,128,TPC]

    # slot matrix
    idxm = np.full((NCORES, 128, slot_cols), ZROW, np.int32)
    i_all = np.arange(NP)
    t_all = i_all // 128
    p_all = i_all % 128
    col_self = off[t_all]
    for c in range(NCORES):
        idxm[c, p_all[:12500], col_self[:12500]] = (c * NP + i_all[:12500]).astype(
            np.int32
        )
    # edge slots grouped by destination dev position
    dkey = pos_dev[dst]
    sval = pos_dev[src].astype(np.int32)
    e_order = np.argsort(dkey, kind="stable")
    dkey = dkey[e_order]
    sval = sval[e_order]
    counts = np.bincount(dkey, minlength=V)
    starts = np.concatenate([[0], np.cumsum(counts)[:-1]])
    jpos = np.arange(E) - starts[dkey] + 1  # slot 0 is self
    ce = dkey // NP
    le = dkey % NP
    te = le // 128
    pe = le % 128
    cols = off[te] + jpos
    idxm[ce, pe, cols] = sval

    # x slices, feature-major
    xs = np.asarray(x, dtype=np.float32)
    xp = np.zeros((NCORES, NP, IN_C), np.float32)
    xp[core_of, lrank] = xs
    xT = np.ascontiguousarray(xp.transpose(0, 2, 1))  # [NC, 128, NP]

    return {
        "xT": xT,
        "deg": deg_tm,
        "idxm": idxm,
        "c_t": c_t,
        "off": off,
        "slot_cols": slot_cols,
        "core_of": core_of,
        "lrank": lrank,
    }


def _build_program(c_t, off, slot_cols):
    nc = bacc.Bacc("TRN2", target_bir_lowering=False, debug=False, num_devices=NCORES)
    f32 = mybir.dt.float32

    xT_d = nc.dram_tensor("xT", [128, NP], f32, kind="ExternalInput")
    deg_d = nc.dram_tensor("deg", [128, TPC], f32, kind="ExternalInput")
    idx_d = nc.dram_tensor("idx", [128, slot_cols], mybir.dt.int32, kind="ExternalInput")
    w1_d = nc.dram_tensor("w1", [IN_C, HID_C], f32, kind="ExternalInput")
    w2_d = nc.dram_tensor("w2", [HID_C, OUT_C], f32, kind="ExternalInput")
    b1_d = nc.dram_tensor("b1", [128, HID_C], f32, kind="ExternalInput")
    b2_d = nc.dram_tensor("b2", [128, OUT_C], f32, kind="ExternalInput")
    out_d = nc.dram_tensor("out", [NP, OUT_C], f32, kind="ExternalOutput")

    h1o = nc.dram_tensor("h1o", [NP, HID_C], f32)          # internal
    tab1 = nc.dram_tensor("tab1", [VT, HID_C], f32)        # internal
    h2o = nc.dram_tensor("h2o", [NP, OUT_C], f32)          # internal
    tab2 = nc.dram_tensor("tab2", [VT, OUT_C], f32)        # internal

    D = HID_C  # == OUT_C == 64
    maxc = int(max(c_t))
    rg = [list(range(NCORES))]

    with tile.TileContext(nc) as tc:
        with (
            tc.tile_pool(name="const", bufs=1) as cpool,
            tc.tile_pool(name="work", bufs=3) as wpool,
            tc.tile_pool(name="gath", bufs=2) as gpool,
            tc.tile_pool(name="psum", bufs=2, space="PSUM") as ppool,
        ):
            # constants
            w1_sb = cpool.tile([IN_C, HID_C], f32)
            nc.sync.dma_start(out=w1_sb[:], in_=w1_d.ap())
            w2_sb = cpool.tile([HID_C, OUT_C], f32)
            nc.sync.dma_start(out=w2_sb[:], in_=w2_d.ap())
            b1_sb = cpool.tile([128, HID_C], f32)
            nc.sync.dma_start(out=b1_sb[:], in_=b1_d.ap())
            b2_sb = cpool.tile([128, OUT_C], f32)
            nc.sync.dma_start(out=b2_sb[:], in_=b2_d.ap())
            idx_sb = cpool.tile([128, slot_cols], mybir.dt.int32)
            nc.sync.dma_start(out=idx_sb[:], in_=idx_d.ap())
            deg_sb = cpool.tile([128, TPC], f32)
            nc.sync.dma_start(out=deg_sb[:], in_=deg_d.ap())
            dinv_sb = cpool.tile([128, TPC], f32)
            rdeg = cpool.tile([128, TPC], f32)
            nc.vector.reciprocal(out=rdeg[:], in_=deg_sb[:])
            nc.scalar.activation(
                out=dinv_sb[:], in_=rdeg[:], func=mybir.ActivationFunctionType.Sqrt
            )
            ident = cpool.tile([128, 128], f32)
            make_identity(nc, ident[:])
            zrow = cpool.tile([1, D], f32)
            nc.vector.memset(zrow[:], 0.0)
            nc.sync.dma_start(out=tab1.ap()[ZROW : ZROW + 1, :], in_=zrow[:])
            nc.sync.dma_start(out=tab2.ap()[ZROW : ZROW + 1, :], in_=zrow[:])

            # phase 1: h1' = dinv * (x @ W1) for own rows
            for t in range(TPC):
                xt = wpool.tile([128, 128], f32, tag="xt")
                nc.sync.dma_start(out=xt[:], in_=xT_d.ap()[:, t * 128 : (t + 1) * 128])
                ps = ppool.tile([128, HID_C], f32, tag="mm")
                nc.tensor.matmul(out=ps[:], lhsT=xt[:], rhs=w1_sb[:], start=True, stop=True)
                h1p = wpool.tile([128, HID_C], f32, tag="h1p")
                nc.vector.tensor_scalar_mul(h1p[:], ps[:], dinv_sb[:, t : t + 1])
                nc.sync.dma_start(out=h1o.ap()[t * 128 : (t + 1) * 128, :], in_=h1p[:])

            nc.gpsimd.collective_compute(
                "AllGather",
                mybir.AluOpType.bypass,
                replica_groups=rg,
                ins=[h1o.ap()],
                outs=[tab1.ap()[0:V, :]],
            )

            # phase 2: aggregate layer 1, relu, matmul W2, scale -> h2o
            for t in range(TPC):
                ct = int(c_t[t])
                o0 = int(off[t])
                gbuf = gpool.tile([128, maxc * D], f32, tag="gbuf")
                for j in range(ct):
                    nc.gpsimd.indirect_dma_start(
                        out=gbuf[:, j * D : (j + 1) * D],
                        out_offset=None,
                        in_=tab1.ap(),
                        in_offset=bass.IndirectOffsetOnAxis(
                            ap=idx_sb[:, o0 + j : o0 + j + 1], axis=0
                        ),
                    )
                s = wpool.tile([128, D], f32, tag="s")
                gv = gbuf[:, : ct * D].rearrange("p (c d) -> p d c", d=D)
                nc.vector.reduce_sum(out=s[:], in_=gv, axis=mybir.AxisListType.X)
                nc.vector.tensor_scalar_mul(s[:], s[:], dinv_sb[:, t : t + 1])
                nc.vector.tensor_add(out=s[:], in0=s[:], in1=b1_sb[:])
                h1t = wpool.tile([128, D], f32, tag="h1t")
                nc.scalar.activation(
                    out=h1t[:], in_=s[:], func=mybir.ActivationFunctionType.Relu
                )
                psT = ppool.tile([HID_C, 128], f32, tag="tr")
                nc.tensor.transpose(out=psT[:], in_=h1t[:], identity=ident[:])
                h1T = wpool.tile([HID_C, 128], f32, tag="h1T")
                nc.vector.tensor_copy(out=h1T[:], in_=psT[:])
                ps2 = ppool.tile([128, OUT_C], f32, tag="mm")
                nc.tensor.matmul(
                    out=ps2[:], lhsT=h1T[:], rhs=w2_sb[:], start=True, stop=True
                )
                h2p = wpool.tile([128, OUT_C], f32, tag="h2p")
                nc.vector.tensor_scalar_mul(h2p[:], ps2[:], dinv_sb[:, t : t + 1])
                nc.sync.dma_start(out=h2o.ap()[t * 128 : (t + 1) * 128, :], in_=h2p[:])

            nc.gpsimd.collective_compute(
                "AllGather",
                mybir.AluOpType.bypass,
                replica_groups=rg,
                ins=[h2o.ap()],
                outs=[tab2.ap()[0:V, :]],
            )

            # phase 4: aggregate layer 2 -> out
            for t in range(TPC):
                ct = int(c_t[t])
                o0 = int(off[t])
                gbuf = gpool.tile([128, maxc * D], f32, tag="gbuf")
                for j in range(ct):
                    nc.gpsimd.indirect_dma_start(
                        out=gbuf[:, j * D : (j + 1) * D],
                        out_offset=None,
                        in_=tab2.ap(),
                        in_offset=bass.IndirectOffsetOnAxis(
                            ap=idx_sb[:, o0 + j : o0 + j + 1], axis=0
                        ),
                    )
                s2 = wpool.tile([128, D], f32, tag="s")
                gv = gbuf[:, : ct * D].rearrange("p (c d) -> p d c", d=D)
                nc.vector.reduce_sum(out=s2[:], in_=gv, axis=mybir.AxisListType.X)
                nc.vector.tensor_scalar_mul(s2[:], s2[:], dinv_sb[:, t : t + 1])
                o2 = wpool.tile([128, OUT_C], f32, tag="h2p")
                nc.vector.tensor_add(out=o2[:], in0=s2[:], in1=b2_sb[:])
                nc.sync.dma_start(out=out_d.ap()[t * 128 : (t + 1) * 128, :], in_=o2[:])

    nc.compile()
    return nc


def kernel(x, edge_index, W1, b1, W2, b2):
    global last_exec_time_ns
    _install_trace_hook()
    sh = _shard(x, edge_index)
    nc = _build_program(sh["c_t"], sh["off"], sh["slot_cols"])

    b1_bc = np.tile(np.asarray(b1, np.float32)[None, :], (128, 1))
    b2_bc = np.tile(np.asarray(b2, np.float32)[None, :], (128, 1))
    w1_np = np.asarray(W1, np.float32)
    w2_np = np.asarray(W2, np.float32)

    in_maps = []
    for c in range(NCORES):
        in_maps.append(
            {
                "xT": sh["xT"][c],
                "deg": sh["deg"][c],
                "idx": sh["idxm"][c],
                "w1": w1_np,
                "w2": w2_np,
                "b1": b1_bc,
                "b2": b2_bc,
            }
        )

    trace = bool(os.environ.get("BASS_TRACE"))
    res = bass_utils.run_bass_kernel_spmd(
        nc, in_maps, core_ids=list(range(NCORES)), trace=trace
    )
    last_exec_time_ns = res.exec_time_ns

    outs = np.stack([res.results[c]["out"] for c in range(NCORES)])  # [NC, NP, 64]
    final = outs[sh["core_of"], sh["lrank"]]
    return final.astype(np.float32)


# revision 3
# speedup vs baseline: 1.0544x; 1.0544x over previous
"""2-layer GCN (GCNConv 128->64->64, symmetric norm, self-loops) on 8 TRN2 cores.

Strategy (graph/data parallel, dst-sharded):
 - Nodes are assigned to the 8 cores by degree-sorted snake dealing, so every
   core holds 12500 nodes and tile t on every core covers the same in-degree
   stratum (SPMD: one program, per-core data).
 - Aggregation identity used: with dinv = rsqrt(1+indeg),
       out[d] = dinv[d] * sum_{e: dst=d} dinv[src_e]*h[src_e]
              + dinv[d]^2 * h[d] + b
   so the table gathered per edge is h' = dinv * h and the self-loop is just
   one more slot. Slot j of node p in tile t holds the table row index of its
   j-th neighbor (slot 0 = self, pads point at a zero row).
 - Per layer: each core computes h' for its own rows (PE matmul), cores
   AllGather the full 100352-row table, then each core gathers its edges'
   rows with [128,1]-index indirect DMAs (one slot column per call) and
   reduces per tile.
"""

import os
import sys
import types

import numpy as np

import concourse.bass as bass
import concourse.bacc as bacc
import concourse.mybir as mybir
import concourse.tile as tile
from concourse import bass_utils
from concourse.masks import make_identity

N = 100000
E = 1600000
IN_C, HID_C, OUT_C = 128, 64, 64
NCORES = 8
TPC = 98                 # tiles per core
NP = TPC * 128           # padded nodes per core = 12544
V = NCORES * NP          # table rows (8 cores' slices) = 100352
ZROW = V                 # index of the all-zero table row
VT = V + 1

last_exec_time_ns = None


def _install_trace_hook():
    """Register the NTFF profile hook missing from this container's antenv stub."""
    if "antenv.axon_hooks" in sys.modules:
        return
    try:
        from trn_agent_boot.trn_boot import _ntff_profile_via_ctypes

        hook = _ntff_profile_via_ctypes("/opt/axon/libaxon_pjrt.so")
        m = types.ModuleType("antenv.axon_hooks")
        m._hook = hook
        m.get_axon_ntff_profile_hook = lambda: m._hook
        m.set_axon_ntff_profile_hook = lambda h: setattr(m, "_hook", h)
        sys.modules["antenv.axon_hooks"] = m
        bass_utils.upload_artifacts = lambda tmpdir: tmpdir
    except Exception:
        pass


def _shard(x, edge_index):
    """Host-side sharding: node->core assignment, permutation, slot matrix."""
    src = np.asarray(edge_index[0], dtype=np.int64)
    dst = np.asarray(edge_index[1], dtype=np.int64)
    indeg = np.bincount(dst, minlength=N)

    order = np.argsort(-indeg, kind="stable")
    snake = np.array([0, 1, 2, 3, 4, 5, 6, 7, 7, 6, 5, 4, 3, 2, 1, 0])
    r = np.arange(N)
    core_of_rank = snake[r % 16]
    local_rank = (r // 16) * 2 + (r % 16 >= 8)
    core_of = np.empty(N, np.int64)
    lrank = np.empty(N, np.int64)
    core_of[order] = core_of_rank
    lrank[order] = local_rank
    pos_dev = core_of * NP + lrank  # dev table row of each original node

    # per-core degree (tile-major [128, TPC]) and slot schedule
    slots_arr = np.zeros((NCORES, NP), np.int64)
    slots_arr[core_of, lrank] = indeg
    c_t = slots_arr.reshape(NCORES, TPC, 128).max(axis=(0, 2)).astype(np.int64)
    c_t = np.maximum(c_t, 1)
    off = np.concatenate([[0], np.cumsum(c_t)]).astype(np.int64)
    slot_cols = int(off[-1])

    deg_arr = np.ones((NCORES, NP), np.float32)
    deg_arr[core_of, lrank] = (1 + indeg).astype(np.float32)
    deg_tm = deg_arr.reshape(NCORES, TPC, 128).transpose(0, 2, 1).copy()  # [NC,128,TPC]

    # slot matrix
    idxm = np.full((NCORES, 128, slot_cols), ZROW, np.int32)
    # edge slots grouped by destination dev position (self handled on-chip)
    dkey = pos_dev[dst]
    sval = pos_dev[src].astype(np.int32)
    e_order = np.argsort(dkey, kind="stable")
    dkey = dkey[e_order]
    sval = sval[e_order]
    counts = np.bincount(dkey, minlength=V)
    starts = np.concatenate([[0], np.cumsum(counts)[:-1]])
    jpos = np.arange(E) - starts[dkey]
    ce = dkey // NP
    le = dkey % NP
    te = le // 128
    pe = le % 128
    cols = off[te] + jpos
    idxm[ce, pe, cols] = sval

    # x slices, feature-major
    xs = np.asarray(x, dtype=np.float32)
    xp = np.zeros((NCORES, NP, IN_C), np.float32)
    xp[core_of, lrank] = xs
    xT = np.ascontiguousarray(xp.transpose(0, 2, 1))  # [NC, 128, NP]

    return {
        "xT": xT,
        "deg": deg_tm,
        "idxm": idxm,
        "c_t": c_t,
        "off": off,
        "slot_cols": slot_cols,
        "core_of": core_of,
        "lrank": lrank,
    }


def _build_program(c_t, off, slot_cols):
    nc = bacc.Bacc("TRN2", target_bir_lowering=False, debug=False, num_devices=NCORES)
    f32 = mybir.dt.float32

    xT_d = nc.dram_tensor("xT", [128, NP], f32, kind="ExternalInput")
    deg_d = nc.dram_tensor("deg", [128, TPC], f32, kind="ExternalInput")
    idx_d = nc.dram_tensor("idx", [128, slot_cols], mybir.dt.int32, kind="ExternalInput")
    w1_d = nc.dram_tensor("w1", [IN_C, HID_C], f32, kind="ExternalInput")
    w2_d = nc.dram_tensor("w2", [HID_C, OUT_C], f32, kind="ExternalInput")
    b1_d = nc.dram_tensor("b1", [128, HID_C], f32, kind="ExternalInput")
    b2_d = nc.dram_tensor("b2", [128, OUT_C], f32, kind="ExternalInput")
    out_d = nc.dram_tensor("out", [NP, OUT_C], f32, kind="ExternalOutput")

    h1o = nc.dram_tensor("h1o", [NP, HID_C], f32)          # internal
    tab1 = nc.dram_tensor("tab1", [VT, HID_C], f32)        # internal
    h2o = nc.dram_tensor("h2o", [NP, OUT_C], f32)          # internal
    tab2 = nc.dram_tensor("tab2", [VT, OUT_C], f32)        # internal

    D = HID_C  # == OUT_C == 64
    maxc = int(max(c_t))
    rg = [list(range(NCORES))]

    with tile.TileContext(nc) as tc:
        with (
            tc.tile_pool(name="const", bufs=1) as cpool,
            tc.tile_pool(name="work", bufs=3) as wpool,
            tc.tile_pool(name="gath", bufs=2) as gpool,
            tc.tile_pool(name="psum", bufs=2, space="PSUM") as ppool,
        ):
            # constants
            w1_sb = cpool.tile([IN_C, HID_C], f32)
            nc.sync.dma_start(out=w1_sb[:], in_=w1_d.ap())
            w2_sb = cpool.tile([HID_C, OUT_C], f32)
            nc.sync.dma_start(out=w2_sb[:], in_=w2_d.ap())
            b1_sb = cpool.tile([128, HID_C], f32)
            nc.sync.dma_start(out=b1_sb[:], in_=b1_d.ap())
            b2_sb = cpool.tile([128, OUT_C], f32)
            nc.sync.dma_start(out=b2_sb[:], in_=b2_d.ap())
            idx_sb = cpool.tile([128, slot_cols], mybir.dt.int32)
            nc.sync.dma_start(out=idx_sb[:], in_=idx_d.ap())
            deg_sb = cpool.tile([128, TPC], f32)
            nc.sync.dma_start(out=deg_sb[:], in_=deg_d.ap())
            dinv_sb = cpool.tile([128, TPC], f32)
            rdeg = cpool.tile([128, TPC], f32)
            nc.vector.reciprocal(out=rdeg[:], in_=deg_sb[:])
            nc.scalar.activation(
                out=dinv_sb[:], in_=rdeg[:], func=mybir.ActivationFunctionType.Sqrt
            )
            ident = cpool.tile([128, 128], f32)
            make_identity(nc, ident[:])
            h1own = cpool.tile([128, TPC * D], f32)
            h2own = cpool.tile([128, TPC * D], f32)
            zrow = cpool.tile([1, D], f32)
            nc.vector.memset(zrow[:], 0.0)
            nc.sync.dma_start(out=tab1.ap()[ZROW : ZROW + 1, :], in_=zrow[:])
            nc.sync.dma_start(out=tab2.ap()[ZROW : ZROW + 1, :], in_=zrow[:])

            # phase 1: h1' = dinv * (x @ W1) for own rows
            for t in range(TPC):
                xt = wpool.tile([128, 128], f32, tag="xt")
                nc.sync.dma_start(out=xt[:], in_=xT_d.ap()[:, t * 128 : (t + 1) * 128])
                ps = ppool.tile([128, HID_C], f32, tag="mm")
                nc.tensor.matmul(out=ps[:], lhsT=xt[:], rhs=w1_sb[:], start=True, stop=True)
                h1p = h1own[:, t * D : (t + 1) * D]
                nc.vector.tensor_scalar_mul(h1p, ps[:], dinv_sb[:, t : t + 1])
                nc.sync.dma_start(out=h1o.ap()[t * 128 : (t + 1) * 128, :], in_=h1p)

            nc.gpsimd.collective_compute(
                "AllGather",
                mybir.AluOpType.bypass,
                replica_groups=rg,
                ins=[h1o.ap()],
                outs=[tab1.ap()[0:V, :]],
            )

            # phase 2: aggregate layer 1, relu, matmul W2, scale -> h2o
            for t in range(TPC):
                ct = int(c_t[t])
                o0 = int(off[t])
                gbuf = gpool.tile([128, maxc * D], f32, tag="gbuf")
                for j in range(ct):
                    nc.gpsimd.indirect_dma_start(
                        out=gbuf[:, j * D : (j + 1) * D],
                        out_offset=None,
                        in_=tab1.ap(),
                        in_offset=bass.IndirectOffsetOnAxis(
                            ap=idx_sb[:, o0 + j : o0 + j + 1], axis=0
                        ),
                    )
                s = wpool.tile([128, D], f32, tag="s")
                gv = gbuf[:, : ct * D].rearrange("p (c d) -> p d c", d=D)
                nc.vector.reduce_sum(out=s[:], in_=gv, axis=mybir.AxisListType.X)
                nc.vector.tensor_add(
                    out=s[:], in0=s[:], in1=h1own[:, t * D : (t + 1) * D]
                )
                nc.vector.tensor_scalar_mul(s[:], s[:], dinv_sb[:, t : t + 1])
                nc.vector.tensor_add(out=s[:], in0=s[:], in1=b1_sb[:])
                h1t = wpool.tile([128, D], f32, tag="h1t")
                nc.scalar.activation(
                    out=h1t[:], in_=s[:], func=mybir.ActivationFunctionType.Relu
                )
                psT = ppool.tile([HID_C, 128], f32, tag="tr")
                nc.tensor.transpose(out=psT[:], in_=h1t[:], identity=ident[:])
                h1T = wpool.tile([HID_C, 128], f32, tag="h1T")
                nc.vector.tensor_copy(out=h1T[:], in_=psT[:])
                ps2 = ppool.tile([128, OUT_C], f32, tag="mm")
                nc.tensor.matmul(
                    out=ps2[:], lhsT=h1T[:], rhs=w2_sb[:], start=True, stop=True
                )
                h2p = h2own[:, t * D : (t + 1) * D]
                nc.vector.tensor_scalar_mul(h2p, ps2[:], dinv_sb[:, t : t + 1])
                nc.sync.dma_start(out=h2o.ap()[t * 128 : (t + 1) * 128, :], in_=h2p)

            nc.gpsimd.collective_compute(
                "AllGather",
                mybir.AluOpType.bypass,
                replica_groups=rg,
                ins=[h2o.ap()],
                outs=[tab2.ap()[0:V, :]],
            )

            # phase 4: aggregate layer 2 -> out
            for t in range(TPC):
                ct = int(c_t[t])
                o0 = int(off[t])
                gbuf = gpool.tile([128, maxc * D], f32, tag="gbuf")
                for j in range(ct):
                    nc.gpsimd.indirect_dma_start(
                        out=gbuf[:, j * D : (j + 1) * D],
                        out_offset=None,
                        in_=tab2.ap(),
                        in_offset=bass.IndirectOffsetOnAxis(
                            ap=idx_sb[:, o0 + j : o0 + j + 1], axis=0
                        ),
                    )
                s2 = wpool.tile([128, D], f32, tag="s")
                gv = gbuf[:, : ct * D].rearrange("p (c d) -> p d c", d=D)
                nc.vector.reduce_sum(out=s2[:], in_=gv, axis=mybir.AxisListType.X)
                nc.vector.tensor_add(
                    out=s2[:], in0=s2[:], in1=h2own[:, t * D : (t + 1) * D]
                )
                nc.vector.tensor_scalar_mul(s2[:], s2[:], dinv_sb[:, t : t + 1])
                o2 = wpool.tile([128, OUT_C], f32, tag="h2p")
                nc.vector.tensor_add(out=o2[:], in0=s2[:], in1=b2_sb[:])
                nc.sync.dma_start(out=out_d.ap()[t * 128 : (t + 1) * 128, :], in_=o2[:])

    nc.compile()
    return nc


def kernel(x, edge_index, W1, b1, W2, b2):
    global last_exec_time_ns
    _install_trace_hook()
    sh = _shard(x, edge_index)
    nc = _build_program(sh["c_t"], sh["off"], sh["slot_cols"])

    b1_bc = np.tile(np.asarray(b1, np.float32)[None, :], (128, 1))
    b2_bc = np.tile(np.asarray(b2, np.float32)[None, :], (128, 1))
    w1_np = np.asarray(W1, np.float32)
    w2_np = np.asarray(W2, np.float32)

    in_maps = []
    for c in range(NCORES):
        in_maps.append(
            {
                "xT": sh["xT"][c],
                "deg": sh["deg"][c],
                "idx": sh["idxm"][c],
                "w1": w1_np,
                "w2": w2_np,
                "b1": b1_bc,
                "b2": b2_bc,
            }
        )

    trace = bool(os.environ.get("BASS_TRACE"))
    res = bass_utils.run_bass_kernel_spmd(
        nc, in_maps, core_ids=list(range(NCORES)), trace=trace
    )
    last_exec_time_ns = res.exec_time_ns

    outs = np.stack([res.results[c]["out"] for c in range(NCORES)])  # [NC, NP, 64]
    final = outs[sh["core_of"], sh["lrank"]]
    return final.astype(np.float32)


# revision 4
# speedup vs baseline: 1.0716x; 1.0163x over previous
"""2-layer GCN (GCNConv 128->64->64, symmetric norm, self-loops) on 8 TRN2 cores.

Strategy (graph/data parallel, dst-sharded):
 - Nodes are assigned to the 8 cores by degree-sorted snake dealing, so every
   core holds 12500 nodes and tile t on every core covers the same in-degree
   stratum (SPMD: one program, per-core data).
 - Aggregation identity used: with dinv = rsqrt(1+indeg),
       out[d] = dinv[d] * sum_{e: dst=d} dinv[src_e]*h[src_e]
              + dinv[d]^2 * h[d] + b
   so the table gathered per edge is h' = dinv * h and the self-loop is just
   one more slot. Slot j of node p in tile t holds the table row index of its
   j-th neighbor (slot 0 = self, pads point at a zero row).
 - Per layer: each core computes h' for its own rows (PE matmul), cores
   AllGather the full 100352-row table, then each core gathers its edges'
   rows with [128,1]-index indirect DMAs (one slot column per call) and
   reduces per tile.
"""

import os
import sys
import types

import numpy as np

import concourse.bass as bass
import concourse.bacc as bacc
import concourse.mybir as mybir
import concourse.tile as tile
from concourse import bass_utils
from concourse.masks import make_identity

N = 100000
E = 1600000
IN_C, HID_C, OUT_C = 128, 64, 64
NCORES = 8
TPC = 98                 # tiles per core
NP = TPC * 128           # padded nodes per core = 12544
V = NCORES * NP          # table rows (8 cores' slices) = 100352
ZROW = V                 # index of the all-zero table row
VT = V + 1

last_exec_time_ns = None


def _install_trace_hook():
    """Register the NTFF profile hook missing from this container's antenv stub."""
    if "antenv.axon_hooks" in sys.modules:
        return
    try:
        from trn_agent_boot.trn_boot import _ntff_profile_via_ctypes

        hook = _ntff_profile_via_ctypes("/opt/axon/libaxon_pjrt.so")
        m = types.ModuleType("antenv.axon_hooks")
        m._hook = hook
        m.get_axon_ntff_profile_hook = lambda: m._hook
        m.set_axon_ntff_profile_hook = lambda h: setattr(m, "_hook", h)
        sys.modules["antenv.axon_hooks"] = m
        bass_utils.upload_artifacts = lambda tmpdir: tmpdir
    except Exception:
        pass


def _shard(x, edge_index):
    """Host-side sharding: node->core assignment, permutation, slot matrix."""
    src = np.asarray(edge_index[0], dtype=np.int64)
    dst = np.asarray(edge_index[1], dtype=np.int64)
    indeg = np.bincount(dst, minlength=N)

    order = np.argsort(-indeg, kind="stable")
    snake = np.array([0, 1, 2, 3, 4, 5, 6, 7, 7, 6, 5, 4, 3, 2, 1, 0])
    r = np.arange(N)
    core_of_rank = snake[r % 16]
    local_rank = (r // 16) * 2 + (r % 16 >= 8)
    core_of = np.empty(N, np.int64)
    lrank = np.empty(N, np.int64)
    core_of[order] = core_of_rank
    lrank[order] = local_rank
    pos_dev = core_of * NP + lrank  # dev table row of each original node

    # per-core degree (tile-major [128, TPC]) and slot schedule
    slots_arr = np.zeros((NCORES, NP), np.int64)
    slots_arr[core_of, lrank] = indeg
    c_t = slots_arr.reshape(NCORES, TPC, 128).max(axis=(0, 2)).astype(np.int64)
    c_t = np.maximum(c_t, 1)
    off = np.concatenate([[0], np.cumsum(c_t)]).astype(np.int64)
    slot_cols = int(off[-1])

    deg_arr = np.ones((NCORES, NP), np.float32)
    deg_arr[core_of, lrank] = (1 + indeg).astype(np.float32)
    deg_tm = deg_arr.reshape(NCORES, TPC, 128).transpose(0, 2, 1).copy()  # [NC,128,TPC]

    # slot matrix
    idxm = np.full((NCORES, 128, slot_cols), ZROW, np.int32)
    # edge slots grouped by destination dev position (self handled on-chip)
    dkey = pos_dev[dst]
    sval = pos_dev[src].astype(np.int32)
    e_order = np.argsort(dkey, kind="stable")
    dkey = dkey[e_order]
    sval = sval[e_order]
    counts = np.bincount(dkey, minlength=V)
    starts = np.concatenate([[0], np.cumsum(counts)[:-1]])
    jpos = np.arange(E) - starts[dkey]
    ce = dkey // NP
    le = dkey % NP
    te = le // 128
    pe = le % 128
    cols = off[te] + jpos
    idxm[ce, pe, cols] = sval

    # x slices, feature-major
    xs = np.asarray(x, dtype=np.float32)
    xp = np.zeros((NCORES, NP, IN_C), np.float32)
    xp[core_of, lrank] = xs
    xT = np.ascontiguousarray(xp.transpose(0, 2, 1))  # [NC, 128, NP]

    return {
        "xT": xT,
        "deg": deg_tm,
        "idxm": idxm,
        "c_t": c_t,
        "off": off,
        "slot_cols": slot_cols,
        "core_of": core_of,
        "lrank": lrank,
    }


def _build_program(c_t, off, slot_cols):
    nc = bacc.Bacc("TRN2", target_bir_lowering=False, debug=False, num_devices=NCORES)
    f32 = mybir.dt.float32

    xT_d = nc.dram_tensor("xT", [128, NP], f32, kind="ExternalInput")
    deg_d = nc.dram_tensor("deg", [128, TPC], f32, kind="ExternalInput")
    idx_d = nc.dram_tensor("idx", [128, slot_cols], mybir.dt.int32, kind="ExternalInput")
    w1_d = nc.dram_tensor("w1", [IN_C, HID_C], f32, kind="ExternalInput")
    w2_d = nc.dram_tensor("w2", [HID_C, OUT_C], f32, kind="ExternalInput")
    b1_d = nc.dram_tensor("b1", [128, HID_C], f32, kind="ExternalInput")
    b2_d = nc.dram_tensor("b2", [128, OUT_C], f32, kind="ExternalInput")
    out_d = nc.dram_tensor("out", [NP, OUT_C], f32, kind="ExternalOutput")

    h1o = nc.dram_tensor("h1o", [NP, HID_C], f32)          # internal
    tab1 = nc.dram_tensor("tab1", [VT, HID_C], f32, addr_space="Shared")
    h2o = nc.dram_tensor("h2o", [NP, OUT_C], f32)          # internal
    tab2 = nc.dram_tensor("tab2", [VT, OUT_C], f32, addr_space="Shared")

    D = HID_C  # == OUT_C == 64
    maxc = int(max(c_t))
    rg = [list(range(NCORES))]

    with tile.TileContext(nc) as tc:
        with (
            tc.tile_pool(name="const", bufs=1) as cpool,
            tc.tile_pool(name="work", bufs=3) as wpool,
            tc.tile_pool(name="gath", bufs=2) as gpool,
            tc.tile_pool(name="psum", bufs=2, space="PSUM") as ppool,
        ):
            # constants
            w1_sb = cpool.tile([IN_C, HID_C], f32)
            nc.sync.dma_start(out=w1_sb[:], in_=w1_d.ap())
            w2_sb = cpool.tile([HID_C, OUT_C], f32)
            nc.sync.dma_start(out=w2_sb[:], in_=w2_d.ap())
            b1_sb = cpool.tile([128, HID_C], f32)
            nc.sync.dma_start(out=b1_sb[:], in_=b1_d.ap())
            b2_sb = cpool.tile([128, OUT_C], f32)
            nc.sync.dma_start(out=b2_sb[:], in_=b2_d.ap())
            idx_sb = cpool.tile([128, slot_cols], mybir.dt.int32)
            nc.sync.dma_start(out=idx_sb[:], in_=idx_d.ap())
            deg_sb = cpool.tile([128, TPC], f32)
            nc.sync.dma_start(out=deg_sb[:], in_=deg_d.ap())
            dinv_sb = cpool.tile([128, TPC], f32)
            rdeg = cpool.tile([128, TPC], f32)
            nc.vector.reciprocal(out=rdeg[:], in_=deg_sb[:])
            nc.scalar.activation(
                out=dinv_sb[:], in_=rdeg[:], func=mybir.ActivationFunctionType.Sqrt
            )
            ident = cpool.tile([128, 128], f32)
            make_identity(nc, ident[:])
            h1own = cpool.tile([128, TPC * D], f32)
            h2own = cpool.tile([128, TPC * D], f32)
            zrow = cpool.tile([1, D], f32)
            nc.vector.memset(zrow[:], 0.0)
            nc.sync.dma_start(out=tab1.ap()[ZROW : ZROW + 1, :], in_=zrow[:])
            nc.sync.dma_start(out=tab2.ap()[ZROW : ZROW + 1, :], in_=zrow[:])

            # phase 1: h1' = dinv * (x @ W1) for own rows
            for t in range(TPC):
                xt = wpool.tile([128, 128], f32, tag="xt")
                nc.sync.dma_start(out=xt[:], in_=xT_d.ap()[:, t * 128 : (t + 1) * 128])
                ps = ppool.tile([128, HID_C], f32, tag="mm")
                nc.tensor.matmul(out=ps[:], lhsT=xt[:], rhs=w1_sb[:], start=True, stop=True)
                h1p = h1own[:, t * D : (t + 1) * D]
                nc.vector.tensor_scalar_mul(h1p, ps[:], dinv_sb[:, t : t + 1])
                nc.sync.dma_start(out=h1o.ap()[t * 128 : (t + 1) * 128, :], in_=h1p)

            nc.gpsimd.collective_compute(
                "AllGather",
                mybir.AluOpType.bypass,
                replica_groups=rg,
                ins=[h1o.ap()],
                outs=[tab1.ap()[0:V, :]],
            )

            # phase 2: aggregate layer 1, relu, matmul W2, scale -> h2o
            for t in range(TPC):
                ct = int(c_t[t])
                o0 = int(off[t])
                gbuf = gpool.tile([128, maxc * D], f32, tag="gbuf")
                for j in range(ct):
                    nc.gpsimd.indirect_dma_start(
                        out=gbuf[:, j * D : (j + 1) * D],
                        out_offset=None,
                        in_=tab1.ap(),
                        in_offset=bass.IndirectOffsetOnAxis(
                            ap=idx_sb[:, o0 + j : o0 + j + 1], axis=0
                        ),
                    )
                s = wpool.tile([128, D], f32, tag="s")
                gv = gbuf[:, : ct * D].rearrange("p (c d) -> p d c", d=D)
                nc.vector.reduce_sum(out=s[:], in_=gv, axis=mybir.AxisListType.X)
                nc.vector.tensor_add(
                    out=s[:], in0=s[:], in1=h1own[:, t * D : (t + 1) * D]
                )
                nc.vector.tensor_scalar_mul(s[:], s[:], dinv_sb[:, t : t + 1])
                nc.vector.tensor_add(out=s[:], in0=s[:], in1=b1_sb[:])
                h1t = wpool.tile([128, D], f32, tag="h1t")
                nc.scalar.activation(
                    out=h1t[:], in_=s[:], func=mybir.ActivationFunctionType.Relu
                )
                psT = ppool.tile([HID_C, 128], f32, tag="tr")
                nc.tensor.transpose(out=psT[:], in_=h1t[:], identity=ident[:])
                h1T = wpool.tile([HID_C, 128], f32, tag="h1T")
                nc.vector.tensor_copy(out=h1T[:], in_=psT[:])
                ps2 = ppool.tile([128, OUT_C], f32, tag="mm")
                nc.tensor.matmul(
                    out=ps2[:], lhsT=h1T[:], rhs=w2_sb[:], start=True, stop=True
                )
                h2p = h2own[:, t * D : (t + 1) * D]
                nc.vector.tensor_scalar_mul(h2p, ps2[:], dinv_sb[:, t : t + 1])
                nc.sync.dma_start(out=h2o.ap()[t * 128 : (t + 1) * 128, :], in_=h2p)

            nc.gpsimd.collective_compute(
                "AllGather",
                mybir.AluOpType.bypass,
                replica_groups=rg,
                ins=[h2o.ap()],
                outs=[tab2.ap()[0:V, :]],
            )

            # phase 4: aggregate layer 2 -> out
            for t in range(TPC):
                ct = int(c_t[t])
                o0 = int(off[t])
                gbuf = gpool.tile([128, maxc * D], f32, tag="gbuf")
                for j in range(ct):
                    nc.gpsimd.indirect_dma_start(
                        out=gbuf[:, j * D : (j + 1) * D],
                        out_offset=None,
                        in_=tab2.ap(),
                        in_offset=bass.IndirectOffsetOnAxis(
                            ap=idx_sb[:, o0 + j : o0 + j + 1], axis=0
                        ),
                    )
                s2 = wpool.tile([128, D], f32, tag="s")
                gv = gbuf[:, : ct * D].rearrange("p (c d) -> p d c", d=D)
                nc.vector.reduce_sum(out=s2[:], in_=gv, axis=mybir.AxisListType.X)
                nc.vector.tensor_add(
                    out=s2[:], in0=s2[:], in1=h2own[:, t * D : (t + 1) * D]
                )
                nc.vector.tensor_scalar_mul(s2[:], s2[:], dinv_sb[:, t : t + 1])
                o2 = wpool.tile([128, OUT_C], f32, tag="h2p")
                nc.vector.tensor_add(out=o2[:], in0=s2[:], in1=b2_sb[:])
                nc.sync.dma_start(out=out_d.ap()[t * 128 : (t + 1) * 128, :], in_=o2[:])

    nc.compile()
    return nc


def kernel(x, edge_index, W1, b1, W2, b2):
    global last_exec_time_ns
    _install_trace_hook()
    sh = _shard(x, edge_index)
    nc = _build_program(sh["c_t"], sh["off"], sh["slot_cols"])

    b1_bc = np.tile(np.asarray(b1, np.float32)[None, :], (128, 1))
    b2_bc = np.tile(np.asarray(b2, np.float32)[None, :], (128, 1))
    w1_np = np.asarray(W1, np.float32)
    w2_np = np.asarray(W2, np.float32)

    in_maps = []
    for c in range(NCORES):
        in_maps.append(
            {
                "xT": sh["xT"][c],
                "deg": sh["deg"][c],
                "idx": sh["idxm"][c],
                "w1": w1_np,
                "w2": w2_np,
                "b1": b1_bc,
                "b2": b2_bc,
            }
        )

    trace = bool(os.environ.get("BASS_TRACE"))
    res = bass_utils.run_bass_kernel_spmd(
        nc, in_maps, core_ids=list(range(NCORES)), trace=trace
    )
    last_exec_time_ns = res.exec_time_ns

    outs = np.stack([res.results[c]["out"] for c in range(NCORES)])  # [NC, NP, 64]
    final = outs[sh["core_of"], sh["lrank"]]
    return final.astype(np.float32)


# revision 5
# speedup vs baseline: 1.0732x; 1.0015x over previous
"""2-layer GCN (GCNConv 128->64->64, symmetric norm, self-loops) on 8 TRN2 cores.

Strategy (graph/data parallel, dst-sharded):
 - Nodes are assigned to the 8 cores by degree-sorted snake dealing, so every
   core holds 12500 nodes and tile t on every core covers the same in-degree
   stratum (SPMD: one program, per-core data).
 - Aggregation identity used: with dinv = rsqrt(1+indeg),
       out[d] = dinv[d] * sum_{e: dst=d} dinv[src_e]*h[src_e]
              + dinv[d]^2 * h[d] + b
   so the table gathered per edge is h' = dinv * h and the self-loop is just
   one more slot. Slot j of node p in tile t holds the table row index of its
   j-th neighbor (slot 0 = self, pads point at a zero row).
 - Per layer: each core computes h' for its own rows (PE matmul), cores
   AllGather the full 100352-row table, then each core gathers its edges'
   rows with [128,1]-index indirect DMAs (one slot column per call) and
   reduces per tile.
"""

import os
import sys
import types

import numpy as np

import concourse.bass as bass
import concourse.bacc as bacc
import concourse.mybir as mybir
import concourse.tile as tile
from concourse import bass_utils
from concourse.masks import make_identity

N = 100000
E = 1600000
IN_C, HID_C, OUT_C = 128, 64, 64
NCORES = 8
TPC = 98                 # tiles per core
NP = TPC * 128           # padded nodes per core = 12544
V = NCORES * NP          # table rows (8 cores' slices) = 100352
ZROW = V                 # index of the all-zero table row
VT = V + 1

last_exec_time_ns = None


def _install_trace_hook():
    """Register the NTFF profile hook missing from this container's antenv stub."""
    if "antenv.axon_hooks" in sys.modules:
        return
    try:
        from trn_agent_boot.trn_boot import _ntff_profile_via_ctypes

        hook = _ntff_profile_via_ctypes("/opt/axon/libaxon_pjrt.so")
        m = types.ModuleType("antenv.axon_hooks")
        m._hook = hook
        m.get_axon_ntff_profile_hook = lambda: m._hook
        m.set_axon_ntff_profile_hook = lambda h: setattr(m, "_hook", h)
        sys.modules["antenv.axon_hooks"] = m
        bass_utils.upload_artifacts = lambda tmpdir: tmpdir
    except Exception:
        pass


def _shard(x, edge_index):
    """Host-side sharding: node->core assignment, permutation, slot matrix."""
    src = np.asarray(edge_index[0], dtype=np.int64)
    dst = np.asarray(edge_index[1], dtype=np.int64)
    indeg = np.bincount(dst, minlength=N)

    order = np.argsort(-indeg, kind="stable")
    snake = np.array([0, 1, 2, 3, 4, 5, 6, 7, 7, 6, 5, 4, 3, 2, 1, 0])
    r = np.arange(N)
    core_of_rank = snake[r % 16]
    local_rank = (r // 16) * 2 + (r % 16 >= 8)
    core_of = np.empty(N, np.int64)
    lrank = np.empty(N, np.int64)
    core_of[order] = core_of_rank
    lrank[order] = local_rank
    pos_dev = core_of * NP + lrank  # dev table row of each original node

    # per-core degree (tile-major [128, TPC]) and slot schedule
    slots_arr = np.zeros((NCORES, NP), np.int64)
    slots_arr[core_of, lrank] = indeg
    c_t = slots_arr.reshape(NCORES, TPC, 128).max(axis=(0, 2)).astype(np.int64)
    c_t = np.maximum(c_t, 1)
    off = np.concatenate([[0], np.cumsum(c_t)]).astype(np.int64)
    slot_cols = int(off[-1])

    deg_arr = np.ones((NCORES, NP), np.float32)
    deg_arr[core_of, lrank] = (1 + indeg).astype(np.float32)
    deg_tm = deg_arr.reshape(NCORES, TPC, 128).transpose(0, 2, 1).copy()  # [NC,128,TPC]

    # slot matrix
    idxm = np.full((NCORES, 128, slot_cols), ZROW, np.int32)
    # edge slots grouped by destination dev position (self handled on-chip)
    dkey = pos_dev[dst]
    sval = pos_dev[src].astype(np.int32)
    e_order = np.argsort(dkey, kind="stable")
    dkey = dkey[e_order]
    sval = sval[e_order]
    counts = np.bincount(dkey, minlength=V)
    starts = np.concatenate([[0], np.cumsum(counts)[:-1]])
    jpos = np.arange(E) - starts[dkey]
    ce = dkey // NP
    le = dkey % NP
    te = le // 128
    pe = le % 128
    cols = off[te] + jpos
    idxm[ce, pe, cols] = sval

    # x slices, feature-major
    xs = np.asarray(x, dtype=np.float32)
    xp = np.zeros((NCORES, NP, IN_C), np.float32)
    xp[core_of, lrank] = xs
    xT = np.ascontiguousarray(xp.transpose(0, 2, 1))  # [NC, 128, NP]

    return {
        "xT": xT,
        "deg": deg_tm,
        "idxm": idxm,
        "c_t": c_t,
        "off": off,
        "slot_cols": slot_cols,
        "core_of": core_of,
        "lrank": lrank,
    }


def _build_program(c_t, off, slot_cols):
    nc = bacc.Bacc("TRN2", target_bir_lowering=False, debug=False, num_devices=NCORES)
    f32 = mybir.dt.float32

    xT_d = nc.dram_tensor("xT", [128, NP], f32, kind="ExternalInput")
    deg_d = nc.dram_tensor("deg", [128, TPC], f32, kind="ExternalInput")
    idx_d = nc.dram_tensor("idx", [128, slot_cols], mybir.dt.int32, kind="ExternalInput")
    w1_d = nc.dram_tensor("w1", [IN_C, HID_C], f32, kind="ExternalInput")
    w2_d = nc.dram_tensor("w2", [HID_C, OUT_C], f32, kind="ExternalInput")
    b1_d = nc.dram_tensor("b1", [128, HID_C], f32, kind="ExternalInput")
    b2_d = nc.dram_tensor("b2", [128, OUT_C], f32, kind="ExternalInput")
    out_d = nc.dram_tensor("out", [NP, OUT_C], f32, kind="ExternalOutput")

    h1o = nc.dram_tensor("h1o", [NP, HID_C], f32)          # internal
    tab1 = nc.dram_tensor("tab1", [VT, HID_C], f32, addr_space="Shared")
    h2o = nc.dram_tensor("h2o", [NP, OUT_C], f32)          # internal
    tab2 = nc.dram_tensor("tab2", [VT, OUT_C], f32, addr_space="Shared")

    D = HID_C  # == OUT_C == 64
    maxc = int(max(c_t))
    rg = [list(range(NCORES))]

    with tile.TileContext(nc) as tc:
        with (
            tc.tile_pool(name="const", bufs=1) as cpool,
            tc.tile_pool(name="work", bufs=4) as wpool,
            tc.tile_pool(name="gath", bufs=4) as gpool,
            tc.tile_pool(name="psum", bufs=2, space="PSUM") as ppool,
        ):
            # constants
            w1_sb = cpool.tile([IN_C, HID_C], f32)
            nc.sync.dma_start(out=w1_sb[:], in_=w1_d.ap())
            w2_sb = cpool.tile([HID_C, OUT_C], f32)
            nc.sync.dma_start(out=w2_sb[:], in_=w2_d.ap())
            b1_sb = cpool.tile([128, HID_C], f32)
            nc.sync.dma_start(out=b1_sb[:], in_=b1_d.ap())
            b2_sb = cpool.tile([128, OUT_C], f32)
            nc.sync.dma_start(out=b2_sb[:], in_=b2_d.ap())
            idx_sb = cpool.tile([128, slot_cols], mybir.dt.int32)
            nc.sync.dma_start(out=idx_sb[:], in_=idx_d.ap())
            deg_sb = cpool.tile([128, TPC], f32)
            nc.sync.dma_start(out=deg_sb[:], in_=deg_d.ap())
            dinv_sb = cpool.tile([128, TPC], f32)
            rdeg = cpool.tile([128, TPC], f32)
            nc.vector.reciprocal(out=rdeg[:], in_=deg_sb[:])
            nc.scalar.activation(
                out=dinv_sb[:], in_=rdeg[:], func=mybir.ActivationFunctionType.Sqrt
            )
            ident = cpool.tile([128, 128], f32)
            make_identity(nc, ident[:])
            h1own = cpool.tile([128, TPC * D], f32)
            h2own = cpool.tile([128, TPC * D], f32)
            zrow = cpool.tile([1, D], f32)
            nc.vector.memset(zrow[:], 0.0)
            nc.sync.dma_start(out=tab1.ap()[ZROW : ZROW + 1, :], in_=zrow[:])
            nc.sync.dma_start(out=tab2.ap()[ZROW : ZROW + 1, :], in_=zrow[:])

            # phase 1: h1' = dinv * (x @ W1) for own rows
            for t in range(TPC):
                xt = wpool.tile([128, 128], f32, tag="xt")
                nc.sync.dma_start(out=xt[:], in_=xT_d.ap()[:, t * 128 : (t + 1) * 128])
                ps = ppool.tile([128, HID_C], f32, tag="mm")
                nc.tensor.matmul(out=ps[:], lhsT=xt[:], rhs=w1_sb[:], start=True, stop=True)
                h1p = h1own[:, t * D : (t + 1) * D]
                nc.vector.tensor_scalar_mul(h1p, ps[:], dinv_sb[:, t : t + 1])
                nc.sync.dma_start(out=h1o.ap()[t * 128 : (t + 1) * 128, :], in_=h1p)

            nc.gpsimd.collective_compute(
                "AllGather",
                mybir.AluOpType.bypass,
                replica_groups=rg,
                ins=[h1o.ap()],
                outs=[tab1.ap()[0:V, :]],
            )

            # phase 2: aggregate layer 1, relu, matmul W2, scale -> h2o
            for t in range(TPC):
                ct = int(c_t[t])
                o0 = int(off[t])
                gbuf = gpool.tile([128, maxc * D], f32, tag="gbuf")
                for j in range(ct):
                    nc.gpsimd.indirect_dma_start(
                        out=gbuf[:, j * D : (j + 1) * D],
                        out_offset=None,
                        in_=tab1.ap(),
                        in_offset=bass.IndirectOffsetOnAxis(
                            ap=idx_sb[:, o0 + j : o0 + j + 1], axis=0
                        ),
                    )
                s = wpool.tile([128, D], f32, tag="s")
                gv = gbuf[:, : ct * D].rearrange("p (c d) -> p d c", d=D)
                nc.vector.reduce_sum(out=s[:], in_=gv, axis=mybir.AxisListType.X)
                nc.vector.tensor_add(
                    out=s[:], in0=s[:], in1=h1own[:, t * D : (t + 1) * D]
                )
                nc.vector.tensor_scalar_mul(s[:], s[:], dinv_sb[:, t : t + 1])
                nc.vector.tensor_add(out=s[:], in0=s[:], in1=b1_sb[:])
                h1t = wpool.tile([128, D], f32, tag="h1t")
                nc.scalar.activation(
                    out=h1t[:], in_=s[:], func=mybir.ActivationFunctionType.Relu
                )
                psT = ppool.tile([HID_C, 128], f32, tag="tr")
                nc.tensor.transpose(out=psT[:], in_=h1t[:], identity=ident[:])
                h1T = wpool.tile([HID_C, 128], f32, tag="h1T")
                nc.vector.tensor_copy(out=h1T[:], in_=psT[:])
                ps2 = ppool.tile([128, OUT_C], f32, tag="mm")
                nc.tensor.matmul(
                    out=ps2[:], lhsT=h1T[:], rhs=w2_sb[:], start=True, stop=True
                )
                h2p = h2own[:, t * D : (t + 1) * D]
                nc.vector.tensor_scalar_mul(h2p, ps2[:], dinv_sb[:, t : t + 1])
                nc.sync.dma_start(out=h2o.ap()[t * 128 : (t + 1) * 128, :], in_=h2p)

            nc.gpsimd.collective_compute(
                "AllGather",
                mybir.AluOpType.bypass,
                replica_groups=rg,
                ins=[h2o.ap()],
                outs=[tab2.ap()[0:V, :]],
            )

            # phase 4: aggregate layer 2 -> out
            for t in range(TPC):
                ct = int(c_t[t])
                o0 = int(off[t])
                gbuf = gpool.tile([128, maxc * D], f32, tag="gbuf")
                for j in range(ct):
                    nc.gpsimd.indirect_dma_start(
                        out=gbuf[:, j * D : (j + 1) * D],
                        out_offset=None,
                        in_=tab2.ap(),
                        in_offset=bass.IndirectOffsetOnAxis(
                            ap=idx_sb[:, o0 + j : o0 + j + 1], axis=0
                        ),
                    )
                s2 = wpool.tile([128, D], f32, tag="s")
                gv = gbuf[:, : ct * D].rearrange("p (c d) -> p d c", d=D)
                nc.vector.reduce_sum(out=s2[:], in_=gv, axis=mybir.AxisListType.X)
                nc.vector.tensor_add(
                    out=s2[:], in0=s2[:], in1=h2own[:, t * D : (t + 1) * D]
                )
                nc.vector.tensor_scalar_mul(s2[:], s2[:], dinv_sb[:, t : t + 1])
                o2 = wpool.tile([128, OUT_C], f32, tag="h2p")
                nc.vector.tensor_add(out=o2[:], in0=s2[:], in1=b2_sb[:])
                nc.sync.dma_start(out=out_d.ap()[t * 128 : (t + 1) * 128, :], in_=o2[:])

    nc.compile()
    return nc


def kernel(x, edge_index, W1, b1, W2, b2):
    global last_exec_time_ns
    _install_trace_hook()
    sh = _shard(x, edge_index)
    nc = _build_program(sh["c_t"], sh["off"], sh["slot_cols"])

    b1_bc = np.tile(np.asarray(b1, np.float32)[None, :], (128, 1))
    b2_bc = np.tile(np.asarray(b2, np.float32)[None, :], (128, 1))
    w1_np = np.asarray(W1, np.float32)
    w2_np = np.asarray(W2, np.float32)

    in_maps = []
    for c in range(NCORES):
        in_maps.append(
            {
                "xT": sh["xT"][c],
                "deg": sh["deg"][c],
                "idx": sh["idxm"][c],
                "w1": w1_np,
                "w2": w2_np,
                "b1": b1_bc,
                "b2": b2_bc,
            }
        )

    trace = bool(os.environ.get("BASS_TRACE"))
    res = bass_utils.run_bass_kernel_spmd(
        nc, in_maps, core_ids=list(range(NCORES)), trace=trace
    )
    last_exec_time_ns = res.exec_time_ns

    outs = np.stack([res.results[c]["out"] for c in range(NCORES)])  # [NC, NP, 64]
    final = outs[sh["core_of"], sh["lrank"]]
    return final.astype(np.float32)


# revision 7
# speedup vs baseline: 1.0918x; 1.0173x over previous
"""2-layer GCN (GCNConv 128->64->64, symmetric norm, self-loops) on 8 TRN2 cores.

Strategy (graph/data parallel, dst-sharded):
 - Nodes are assigned to the 8 cores by degree-sorted snake dealing, so every
   core holds 12500 nodes and tile t on every core covers the same in-degree
   stratum (SPMD: one program, per-core data).
 - Aggregation identity used: with dinv = rsqrt(1+indeg),
       out[d] = dinv[d] * sum_{e: dst=d} dinv[src_e]*h[src_e]
              + dinv[d]^2 * h[d] + b
   so the table gathered per edge is h' = dinv * h and the self-loop is just
   one more slot. Slot j of node p in tile t holds the table row index of its
   j-th neighbor (slot 0 = self, pads point at a zero row).
 - Per layer: each core computes h' for its own rows (PE matmul), cores
   AllGather the full 100352-row table, then each core gathers its edges'
   rows with [128,1]-index indirect DMAs (one slot column per call) and
   reduces per tile.
"""

import os
import sys
import types

import numpy as np

import concourse.bass as bass
import concourse.bacc as bacc
import concourse.mybir as mybir
import concourse.tile as tile
from concourse import bass_utils
from concourse.masks import make_identity

N = 100000
E = 1600000
IN_C, HID_C, OUT_C = 128, 64, 64
NCORES = 8
TPC = 98                 # tiles per core
NP = TPC * 128           # padded nodes per core = 12544
V = NCORES * NP          # table rows (8 cores' slices) = 100352
ZROW = V                 # index of the all-zero table row
VT = V + 1

last_exec_time_ns = None


def _install_trace_hook():
    """Register the NTFF profile hook missing from this container's antenv stub."""
    if "antenv.axon_hooks" in sys.modules:
        return
    try:
        from trn_agent_boot.trn_boot import _ntff_profile_via_ctypes

        hook = _ntff_profile_via_ctypes("/opt/axon/libaxon_pjrt.so")
        m = types.ModuleType("antenv.axon_hooks")
        m._hook = hook
        m.get_axon_ntff_profile_hook = lambda: m._hook
        m.set_axon_ntff_profile_hook = lambda h: setattr(m, "_hook", h)
        sys.modules["antenv.axon_hooks"] = m
        bass_utils.upload_artifacts = lambda tmpdir: tmpdir
    except Exception:
        pass


def _shard(x, edge_index):
    """Host-side sharding: node->core assignment, permutation, slot matrix."""
    src = np.asarray(edge_index[0], dtype=np.int64)
    dst = np.asarray(edge_index[1], dtype=np.int64)
    indeg = np.bincount(dst, minlength=N)

    order = np.argsort(-indeg, kind="stable")
    snake = np.array([0, 1, 2, 3, 4, 5, 6, 7, 7, 6, 5, 4, 3, 2, 1, 0])
    r = np.arange(N)
    core_of_rank = snake[r % 16]
    local_rank = (r // 16) * 2 + (r % 16 >= 8)
    core_of = np.empty(N, np.int64)
    lrank = np.empty(N, np.int64)
    core_of[order] = core_of_rank
    lrank[order] = local_rank
    half = NP // 2
    pos_dev = np.where(
        lrank < half,
        core_of * half + lrank,
        NCORES * half + core_of * half + (lrank - half),
    )  # dev table row of each original node (half-split AllGather layout)

    # per-core degree (tile-major [128, TPC]) and slot schedule
    slots_arr = np.zeros((NCORES, NP), np.int64)
    slots_arr[core_of, lrank] = indeg
    c_t = slots_arr.reshape(NCORES, TPC, 128).max(axis=(0, 2)).astype(np.int64)
    c_t = np.maximum(c_t, 1)
    off = np.concatenate([[0], np.cumsum(c_t)]).astype(np.int64)
    slot_cols = int(off[-1])

    deg_arr = np.ones((NCORES, NP), np.float32)
    deg_arr[core_of, lrank] = (1 + indeg).astype(np.float32)
    deg_tm = deg_arr.reshape(NCORES, TPC, 128).transpose(0, 2, 1).copy()  # [NC,128,TPC]

    # slot matrix
    idxm = np.full((NCORES, 128, slot_cols), ZROW, np.int32)
    # edge slots grouped by destination local position (self handled on-chip);
    # slot VALUES use the AllGather table layout (pos_dev), grouping uses the
    # core-local layout.
    gkey = core_of[dst] * NP + lrank[dst]
    sval = pos_dev[src].astype(np.int32)
    e_order = np.argsort(gkey, kind="stable")
    gkey = gkey[e_order]
    sval = sval[e_order]
    counts = np.bincount(gkey, minlength=V)
    starts = np.concatenate([[0], np.cumsum(counts)[:-1]])
    jpos = np.arange(E) - starts[gkey]
    ce = gkey // NP
    le = gkey % NP
    te = le // 128
    pe = le % 128
    cols = off[te] + jpos
    idxm[ce, pe, cols] = sval

    # x slices, feature-major
    xs = np.asarray(x, dtype=np.float32)
    xp = np.zeros((NCORES, NP, IN_C), np.float32)
    xp[core_of, lrank] = xs
    xT = np.ascontiguousarray(xp.transpose(0, 2, 1))  # [NC, 128, NP]

    return {
        "xT": xT,
        "deg": deg_tm,
        "idxm": idxm,
        "c_t": c_t,
        "off": off,
        "slot_cols": slot_cols,
        "core_of": core_of,
        "lrank": lrank,
    }


def _build_program(c_t, off, slot_cols):
    nc = bacc.Bacc("TRN2", target_bir_lowering=False, debug=False, num_devices=NCORES)
    f32 = mybir.dt.float32

    xT_d = nc.dram_tensor("xT", [128, NP], f32, kind="ExternalInput")
    deg_d = nc.dram_tensor("deg", [128, TPC], f32, kind="ExternalInput")
    idx_d = nc.dram_tensor("idx", [128, slot_cols], mybir.dt.int32, kind="ExternalInput")
    w1_d = nc.dram_tensor("w1", [IN_C, HID_C], f32, kind="ExternalInput")
    w2_d = nc.dram_tensor("w2", [HID_C, OUT_C], f32, kind="ExternalInput")
    b1_d = nc.dram_tensor("b1", [128, HID_C], f32, kind="ExternalInput")
    b2_d = nc.dram_tensor("b2", [128, OUT_C], f32, kind="ExternalInput")
    out_d = nc.dram_tensor("out", [NP, OUT_C], f32, kind="ExternalOutput")

    h1o = nc.dram_tensor("h1o", [NP, HID_C], f32)          # internal
    tab1 = nc.dram_tensor("tab1", [VT, HID_C], f32, addr_space="Shared")
    h2o = nc.dram_tensor("h2o", [NP, OUT_C], f32)          # internal
    tab2 = nc.dram_tensor("tab2", [VT, OUT_C], f32, addr_space="Shared")

    D = HID_C  # == OUT_C == 64
    maxc = int(max(c_t))
    rg = [list(range(NCORES))]

    with tile.TileContext(nc) as tc:
        with (
            tc.tile_pool(name="const", bufs=1) as cpool,
            tc.tile_pool(name="work", bufs=4) as wpool,
            tc.tile_pool(name="gath", bufs=4) as gpool,
            tc.tile_pool(name="psum", bufs=2, space="PSUM") as ppool,
        ):
            # constants
            w1_sb = cpool.tile([IN_C, HID_C], f32)
            nc.sync.dma_start(out=w1_sb[:], in_=w1_d.ap())
            w2_sb = cpool.tile([HID_C, OUT_C], f32)
            nc.sync.dma_start(out=w2_sb[:], in_=w2_d.ap())
            b1_sb = cpool.tile([128, HID_C], f32)
            nc.sync.dma_start(out=b1_sb[:], in_=b1_d.ap())
            b2_sb = cpool.tile([128, OUT_C], f32)
            nc.sync.dma_start(out=b2_sb[:], in_=b2_d.ap())
            idx_sb = cpool.tile([128, slot_cols], mybir.dt.int32)
            nc.sync.dma_start(out=idx_sb[:], in_=idx_d.ap())
            deg_sb = cpool.tile([128, TPC], f32)
            nc.sync.dma_start(out=deg_sb[:], in_=deg_d.ap())
            dinv_sb = cpool.tile([128, TPC], f32)
            rdeg = cpool.tile([128, TPC], f32)
            nc.vector.reciprocal(out=rdeg[:], in_=deg_sb[:])
            nc.scalar.activation(
                out=dinv_sb[:], in_=rdeg[:], func=mybir.ActivationFunctionType.Sqrt
            )
            ident = cpool.tile([128, 128], f32)
            make_identity(nc, ident[:])
            h1own = cpool.tile([128, TPC * D], f32)
            h2own = cpool.tile([128, TPC * D], f32)
            zrow = cpool.tile([1, D], f32)
            nc.vector.memset(zrow[:], 0.0)
            nc.sync.dma_start(out=tab1.ap()[ZROW : ZROW + 1, :], in_=zrow[:])
            nc.sync.dma_start(out=tab2.ap()[ZROW : ZROW + 1, :], in_=zrow[:])

            # phase 1: h1' = dinv * (x @ W1) for own rows (x loaded 4 tiles/DMA)
            XB = 4
            for t0 in range(0, TPC, XB):
                nb = min(XB, TPC - t0)
                xt = wpool.tile([128, XB * 128], f32, tag="xt")
                nc.sync.dma_start(
                    out=xt[:, : nb * 128],
                    in_=xT_d.ap()[:, t0 * 128 : (t0 + nb) * 128],
                )
                for k in range(nb):
                    t = t0 + k
                    ps = ppool.tile([128, HID_C], f32, tag="mm")
                    nc.tensor.matmul(
                        out=ps[:],
                        lhsT=xt[:, k * 128 : (k + 1) * 128],
                        rhs=w1_sb[:],
                        start=True,
                        stop=True,
                    )
                    h1p = h1own[:, t * D : (t + 1) * D]
                    nc.vector.tensor_scalar_mul(h1p, ps[:], dinv_sb[:, t : t + 1])
                    nc.sync.dma_start(
                        out=h1o.ap()[t * 128 : (t + 1) * 128, :], in_=h1p
                    )

            half = NP // 2
            nc.gpsimd.collective_compute(
                "AllGather",
                mybir.AluOpType.bypass,
                replica_groups=rg,
                ins=[h1o.ap()[0:half, :]],
                outs=[tab1.ap()[0 : NCORES * half, :]],
            )
            nc.gpsimd.collective_compute(
                "AllGather",
                mybir.AluOpType.bypass,
                replica_groups=rg,
                ins=[h1o.ap()[half:NP, :]],
                outs=[tab1.ap()[NCORES * half : V, :]],
            )

            # phase 2: aggregate layer 1, relu, matmul W2, scale -> h2o
            for t in range(TPC):
                ct = int(c_t[t])
                o0 = int(off[t])
                gbuf = gpool.tile([128, maxc * D], f32, tag="gbuf")
                for j in range(ct):
                    nc.gpsimd.indirect_dma_start(
                        out=gbuf[:, j * D : (j + 1) * D],
                        out_offset=None,
                        in_=tab1.ap(),
                        in_offset=bass.IndirectOffsetOnAxis(
                            ap=idx_sb[:, o0 + j : o0 + j + 1], axis=0
                        ),
                    )
                s = wpool.tile([128, D], f32, tag="s")
                gv = gbuf[:, : ct * D].rearrange("p (c d) -> p d c", d=D)
                nc.vector.reduce_sum(out=s[:], in_=gv, axis=mybir.AxisListType.X)
                nc.vector.tensor_add(
                    out=s[:], in0=s[:], in1=h1own[:, t * D : (t + 1) * D]
                )
                nc.vector.tensor_scalar_mul(s[:], s[:], dinv_sb[:, t : t + 1])
                nc.vector.tensor_add(out=s[:], in0=s[:], in1=b1_sb[:])
                h1t = wpool.tile([128, D], f32, tag="h1t")
                nc.scalar.activation(
                    out=h1t[:], in_=s[:], func=mybir.ActivationFunctionType.Relu
                )
                psT = ppool.tile([HID_C, 128], f32, tag="tr")
                nc.tensor.transpose(out=psT[:], in_=h1t[:], identity=ident[:])
                h1T = wpool.tile([HID_C, 128], f32, tag="h1T")
                nc.vector.tensor_copy(out=h1T[:], in_=psT[:])
                ps2 = ppool.tile([128, OUT_C], f32, tag="mm")
                nc.tensor.matmul(
                    out=ps2[:], lhsT=h1T[:], rhs=w2_sb[:], start=True, stop=True
                )
                h2p = h2own[:, t * D : (t + 1) * D]
                nc.vector.tensor_scalar_mul(h2p, ps2[:], dinv_sb[:, t : t + 1])
                nc.sync.dma_start(out=h2o.ap()[t * 128 : (t + 1) * 128, :], in_=h2p)

            nc.gpsimd.collective_compute(
                "AllGather",
                mybir.AluOpType.bypass,
                replica_groups=rg,
                ins=[h2o.ap()[0:half, :]],
                outs=[tab2.ap()[0 : NCORES * half, :]],
            )
            nc.gpsimd.collective_compute(
                "AllGather",
                mybir.AluOpType.bypass,
                replica_groups=rg,
                ins=[h2o.ap()[half:NP, :]],
                outs=[tab2.ap()[NCORES * half : V, :]],
            )

            # phase 4: aggregate layer 2 -> out
            for t in range(TPC):
                ct = int(c_t[t])
                o0 = int(off[t])
                gbuf = gpool.tile([128, maxc * D], f32, tag="gbuf")
                for j in range(ct):
                    nc.gpsimd.indirect_dma_start(
                        out=gbuf[:, j * D : (j + 1) * D],
                        out_offset=None,
                        in_=tab2.ap(),
                        in_offset=bass.IndirectOffsetOnAxis(
                            ap=idx_sb[:, o0 + j : o0 + j + 1], axis=0
                        ),
                    )
                s2 = wpool.tile([128, D], f32, tag="s")
                gv = gbuf[:, : ct * D].rearrange("p (c d) -> p d c", d=D)
                nc.vector.reduce_sum(out=s2[:], in_=gv, axis=mybir.AxisListType.X)
                nc.vector.tensor_add(
                    out=s2[:], in0=s2[:], in1=h2own[:, t * D : (t + 1) * D]
                )
                nc.vector.tensor_scalar_mul(s2[:], s2[:], dinv_sb[:, t : t + 1])
                o2 = wpool.tile([128, OUT_C], f32, tag="h2p")
                nc.vector.tensor_add(out=o2[:], in0=s2[:], in1=b2_sb[:])
                nc.sync.dma_start(out=out_d.ap()[t * 128 : (t + 1) * 128, :], in_=o2[:])

    nc.compile()
    return nc


def kernel(x, edge_index, W1, b1, W2, b2):
    global last_exec_time_ns
    _install_trace_hook()
    sh = _shard(x, edge_index)
    nc = _build_program(sh["c_t"], sh["off"], sh["slot_cols"])

    b1_bc = np.tile(np.asarray(b1, np.float32)[None, :], (128, 1))
    b2_bc = np.tile(np.asarray(b2, np.float32)[None, :], (128, 1))
    w1_np = np.asarray(W1, np.float32)
    w2_np = np.asarray(W2, np.float32)

    in_maps = []
    for c in range(NCORES):
        in_maps.append(
            {
                "xT": sh["xT"][c],
                "deg": sh["deg"][c],
                "idx": sh["idxm"][c],
                "w1": w1_np,
                "w2": w2_np,
                "b1": b1_bc,
                "b2": b2_bc,
            }
        )

    trace = bool(os.environ.get("BASS_TRACE"))
    res = bass_utils.run_bass_kernel_spmd(
        nc, in_maps, core_ids=list(range(NCORES)), trace=trace
    )
    last_exec_time_ns = res.exec_time_ns

    outs = np.stack([res.results[c]["out"] for c in range(NCORES)])  # [NC, NP, 64]
    final = outs[sh["core_of"], sh["lrank"]]
    return final.astype(np.float32)
